# revision 69
# baseline (speedup 1.0000x reference)
"""Trainium2 Bass kernel for nn_Adapter (audio conv encoder + cross-attention).

Data-parallel over batch: 16 batches / 8 NeuronCores = 2 per core, no
collectives. All heavy matmuls run in bf16 (1 cycle/row on the PE array);
PSUM accumulation is fp32 throughout. Output is written bf16 and upcast on
host.

v2 vs v1:
 - audio encoder processes both batches concurrently (column-tiled convs,
   batch-packed transposes / k / v projections), entirely up front.
 - q-projection's half-wide third m-tile is computed for two token chunks
   at once via column tiling.
 - startup DMAs are spread across the sync/scalar HWDGE rings and the
   gpsimd SWDGE ring in dependency order; context chunks are host-packed
   so each chunk tile is one contiguous DMA.
 - output tensor is bf16.
"""
import sys
sys.path.insert(0, "/opt/trn_rl_repo")

import numpy as np
import ml_dtypes

import concourse.bass as bass
import concourse.mybir as mybir
import concourse.tile as tile
from concourse.bass_utils import run_bass_kernel_spmd

F32 = mybir.dt.float32
BF16 = mybir.dt.bfloat16
AF = mybir.ActivationFunctionType
BF = ml_dtypes.bfloat16

NCORES = 8
B, N, CTX = 16, 4096, 768
BP = B // NCORES            # batches per core
H, D, INNER = 8, 40, 320    # heads, dim_head, inner
AUD = 1024                  # audio feature length
KS, PAD = 17, 8
EPS = 1e-5
SCALE = D ** -0.5
TCH = 512                   # token chunk
NCH = N // TCH              # chunks per batch
PADB = AUD + 2 * PAD

# pair -> sim matmul plan: (kp_tile_index, qt_chunk)
SIM_PLAN = [
    [(0, 0)],           # pair 0 (h0,h1): KP01 x qt_ch0
    [(1, 0), (2, 1)],   # pair 1 (h2,h3): KP23a x ch0 + KP23b x ch1
    [(3, 1)],           # pair 2 (h4,h5): KP45 x ch1
    [(4, 1), (5, 2)],   # pair 3 (h6,h7): KP67a x ch1 + KP67b x ch2
]
KP_DEF = [(0, 0), (0, 1), (1, 1), (1, 2), (1, 3), (2, 3)]  # tile -> (chunk, pair)
ME = [128, 128, 64]                                         # e-chunk sizes

# attention-value (at) tile row layout: e-dims distributed over 3 tiles.
# The 8 softmax denominators sit at rows 64:68 (a 64-aligned base, required
# by the partition-start rules) of T0 (heads 0-3) and T1 (heads 4-7); the
# out-proj bias ones-row is T2 row 64.  Each at tile is produced by
# contracting only the es pairs it touches: T0: p0,p1  T1: p1,p2,p3  T2: p3
# (6 matmuls).  Normalization broadcasts come from two reciprocal tiles
# (recA: heads 0-3, recB: heads 4-7) -> 1+2+1 = 4 broadcast matmuls.
AV_PLAN = {0: [0, 1], 1: [1, 2, 3], 2: [3]}
ATW = [128, 128, 73]            # at-tile matmul/emit widths
BW = [128, 128, 73]             # normalize widths
NORM_SRC = {0: ["a"], 1: ["a", "b"], 2: ["b"]}   # which rec tiles per at tile
# vp build ops: (tile n, pair p, dst col0, src e-dim d0, width)
VP_OPS = [
    (0, 0, 0, 0, 64),
    (0, 0, 68, 64, 16),
    (0, 1, 84, 80, 44),
    (1, 1, 0, 124, 36),
    (1, 2, 36, 160, 28),
    (1, 2, 68, 188, 52),
    (1, 3, 120, 240, 8),
    (2, 3, 0, 248, 64),
    (2, 3, 65, 312, 8),
]
# denominator ones columns: (tile n, pair p, col, head)
DEN_SETS = [
    (0, 0, 64, 0), (0, 0, 65, 1), (0, 1, 66, 2), (0, 1, 67, 3),
    (1, 2, 64, 4), (1, 2, 65, 5), (1, 3, 66, 6), (1, 3, 67, 7),
]


def _head_of(e):
    return e // D


def _at_row_dim(n, r):
    """at-tile row -> e-dim, or None for denominator/bias rows."""
    if n == 0:
        if 64 <= r < 68:
            return None
        return r if r < 64 else r - 4
    if n == 1:
        if 64 <= r < 68:
            return None
        return 124 + r if r < 64 else 120 + r
    if r < 64:
        return 248 + r
    return None if r == 64 else 247 + r


def _build_host_consts(inputs):
    c = {}
    w1, b1 = inputs["w1"], inputs["b1"]
    w2, b2 = inputs["w2"], inputs["b2"]
    w3, b3 = inputs["w3"], inputs["b3"]
    c["w1t"] = np.ascontiguousarray(w1[:, 0, :].T).astype(BF)             # [17, 64]

    def pack_pairs(w):  # [64co, 64ci, 17] -> [128, 9, 64co]
        wp = np.zeros((128, 9, 64), np.float32)
        wt = w.transpose(1, 2, 0)  # [ci, k, co]
        for q in range(9):
            wp[0:64, q, :] = wt[:, 2 * q, :]
            if 2 * q + 1 < KS:
                wp[64:128, q, :] = wt[:, 2 * q + 1, :]
        return wp.astype(BF)

    c["w2p"] = pack_pairs(w2)
    c["w2pB"] = np.concatenate([c["w2p"][64:128], c["w2p"][0:64]], axis=0)
    c["w3p"] = pack_pairs(w3)
    c["w3pB"] = np.concatenate([c["w3p"][64:128], c["w3p"][0:64]], axis=0)
    c["b1c2"] = np.tile(np.asarray(b1).reshape(64, 1), (2, 1)).astype(np.float32)
    c["b2c2"] = np.tile(np.asarray(b2).reshape(64, 1), (2, 1)).astype(np.float32)
    c["b3c2"] = np.tile(np.asarray(b3).reshape(64, 1), (2, 1)).astype(np.float32)
    c["lnw2"] = np.tile(np.asarray(inputs["ln_w"]), (2, 1)).astype(np.float32)
    c["lnb2"] = np.tile(np.asarray(inputs["ln_b"]), (2, 1)).astype(np.float32)

    # cols 320:384 stay zero: the unused array half multiplies by zeros,
    # which keeps PE switching power down (a packed non-zero duplicate was
    # measured to downclock the whole chip to 2.0 GHz)
    wqt = np.zeros((CTX, 384), np.float32)
    wqt[:, :INNER] = np.asarray(inputs["wq"]).T
    c["wqt"] = wqt.astype(BF)
    wkt = np.zeros((AUD, 384), np.float32)
    wkt[:, :INNER] = np.asarray(inputs["wk"]).T
    c["wkt"] = wkt.astype(BF)
    c["wvt"] = np.ascontiguousarray(np.asarray(inputs["wv"]).T).astype(BF)
    woutT = np.asarray(inputs["w_out"]).T          # [320 e, 768]
    wout = np.zeros((384, CTX), np.float32)
    for n in range(3):
        for r in range(ATW[n]):
            d = _at_row_dim(n, r)
            if d is not None:
                wout[128 * n + r] = woutT[d]
    wout[128 * 2 + 64] = np.asarray(inputs["b_out"])
    c["woutA"] = wout.astype(BF)

    km = np.zeros((128, 6, 128), np.float32)
    for t, (n, p) in enumerate(KP_DEF):
        for r in range(ME[n]):
            h = _head_of(128 * n + r)
            if h == 2 * p:
                km[r, t, 0:64] = 1.0
            elif h == 2 * p + 1:
                km[r, t, 64:128] = 1.0
    c["kmask"] = km.astype(BF)

    vm = np.zeros((128, 9, 80), np.float32)
    for si, (n, p, c0, d0, w) in enumerate(VP_OPS):
        for cc in range(w):
            h = _head_of(d0 + cc)
            vm[64 * (h % 2):64 * (h % 2) + 64, si, cc] = 1.0
    c["vm4"] = vm.astype(BF)

    e8a = np.zeros((68, 3, 128), np.float32)
    e8b = np.zeros((68, 3, 128), np.float32)
    for n in range(3):
        for r in range(ATW[n]):
            d = _at_row_dim(n, r)
            if d is not None:
                h = _head_of(d)
                if h < 4:
                    e8a[64 + h, n, r] = 1.0
                else:
                    e8b[60 + h, n, r] = 1.0
    c["exp8a"] = e8a.astype(BF)
    c["exp8b"] = e8b.astype(BF)

    c["ident"] = np.eye(128, dtype=np.float32).astype(BF)
    return c


def _build_graph():
    nc = bass.Bass()
    P = {}

    def inp(name, shape, dt):
        P[name] = nc.declare_dram_parameter(name, list(shape), dt, isOutput=False)

    inp("ctxp", (BP, NCH, 128, 6, TCH), BF16)
    inp("a_im", (BP, KS, AUD), BF16)
    inp("w1t", (KS, 64), BF16)
    inp("w2p", (128, 9, 64), BF16)
    inp("w2pB", (128, 9, 64), BF16)
    inp("w3p", (128, 9, 64), BF16)
    inp("w3pB", (128, 9, 64), BF16)
    inp("b1c2", (128, 1), F32)
    inp("b2c2", (128, 1), F32)
    inp("b3c2", (128, 1), F32)
    inp("lnw2", (128, AUD), F32)
    inp("lnb2", (128, AUD), F32)
    inp("wqt", (CTX, 384), BF16)
    inp("wkt", (AUD, 384), BF16)
    inp("wvt", (AUD, INNER), BF16)
    inp("woutA", (384, CTX), BF16)
    inp("kmask", (128, 6, 128), BF16)
    inp("vm4", (128, 9, 80), BF16)
    inp("exp8a", (68, 3, 128), BF16)
    inp("exp8b", (68, 3, 128), BF16)
    inp("ident", (128, 128), BF16)
    out_e = nc.declare_dram_parameter("out", [BP, N, CTX], BF16, isOutput=True)

    with tile.TileContext(nc) as tc:
        cp = tc.alloc_tile_pool(name="const", bufs=1)
        pp = tc.alloc_tile_pool(name="persist", bufs=1)
        cinp = tc.alloc_tile_pool(name="cinp", bufs=3)
        esp = tc.alloc_tile_pool(name="esp", bufs=6)
        qtp = tc.alloc_tile_pool(name="qtp", bufs=5)
        mp = tc.alloc_tile_pool(name="mp", bufs=2)
        ofp = tc.alloc_tile_pool(name="ofp", bufs=4)
        ap = tc.alloc_tile_pool(name="audio", bufs=1)
        aps = tc.alloc_tile_pool(name="aps", bufs=2, space="PSUM")

        def cload(eng, name, shape, dt, ap_src=None):
            t = cp.tile(list(shape), dt, tag=name)
            eng.dma_start(t[:], ap_src if ap_src is not None else P[name][:])
            return t

        # ---- startup DMAs ----
        # HWDGE dma issues occupy the issuing engine's stream, so the
        # scalar (ACT) ring must stay nearly empty before the first gelu;
        # later consts are emitted mid-preamble at points where that
        # engine idles anyway.
        # warm the ACT function tables before anything depends on ACT
        dummy = cp.tile([1, 4], F32, tag="dummy")
        nc.vector.memset(dummy[:], 1.0)
        nc.scalar.activation(dummy[:], dummy[:], AF.Gelu)
        nc.scalar.activation(dummy[:], dummy[:], AF.Ln)
        nc.scalar.activation(dummy[:], dummy[:], AF.Exp)

        w1t = cload(nc.sync, "w1t", (KS, 64), BF16)
        a_sbs = {}
        for b in range(BP):
            t = ap.tile([KS, AUD], BF16, tag=f"a_im{b}")
            nc.sync.dma_start(t[:], P["a_im"][b])
            a_sbs[b] = t
        b1c2 = cload(nc.sync, "b1c2", (128, 1), F32)
        w2p = cload(nc.sync, "w2p", (128, 9, 64), BF16)
        w2pB = cload(nc.sync, "w2pB", (128, 9, 64), BF16)
        b2c2 = cload(nc.sync, "b2c2", (128, 1), F32)

        # scalar ring: only the q weights early (first ACT op is gelu)
        wqt = cload(nc.scalar, "wqt", (128, 6, 384), BF16,
                    P["wqt"][:].rearrange("(n p) e -> p n e", p=128))

        # gpsimd ring: first two context chunks (more are loaded mid-preamble)
        cins = {}
        cin_loaded = set()

        def load_cin(j):
            t = cinp.tile([128, 6, TCH], BF16, tag="cin", name=f"cin{j}")
            nc.gpsimd.dma_start(t[:], P["ctxp"][j // NCH, j % NCH])
            cins[(j // NCH, j % NCH)] = t
            cin_loaded.add((j // NCH, j % NCH))

        load_cin(0)
        load_cin(1)

        # declared here, loaded mid-preamble (see audio phase emission)
        w3p = w3pB = b3c2 = ident = lnw2 = lnb2 = None
        wkt = wvt = woutA = kmask = vm4 = exp8a = exp8b = None

        # block-diagonal ones for the pair layernorm reduction
        ones2 = cp.tile([128, 128], BF16, tag="ones2")
        nc.vector.memset(ones2[:], 0.0)
        nc.vector.memset(ones2[0:64, 0:64], 1.0)
        nc.vector.memset(ones2[64:128, 64:128], 1.0)

        # ---- q emission helper ----
        qts = {}

        def emit_q(i, psum_pool, psum_tag):
            cin = cins.pop((i // NCH, i % NCH))
            qt = qtp.tile([128, 3, TCH], BF16, tag="qt")
            for m in range(3):
                qp = psum_pool.tile([128, TCH], F32, tag=psum_tag)
                for n6 in range(6):
                    nc.tensor.matmul(qp[:], wqt[:, n6, 128 * m:128 * m + 128],
                                     cin[:, n6, :], start=(n6 == 0), stop=(n6 == 5))
                nc.vector.tensor_copy(qt[:, m, :], qp[:])
            qts[i] = qt
            return qt

        # ---- audio encoder, both batches at once ----
        # batch 0 layout: unshifted rows 0:64, shifted rows 64:128
        # batch 1 layout: mirrored (weights w2pB/w3pB are row-swapped)
        xb2s, xb3s = {}, {}
        for b in range(BP):
            xb2s[b] = ap.tile([128, PADB], BF16, tag=f"xb2{b}",
                              name=f"xb2_{b}")
            xb3s[b] = ap.tile([128, PADB], BF16, tag=f"xb3{b}",
                              name=f"xb3_{b}")
        u0, s0 = slice(0, 64), slice(64, 128)      # batch0 unshifted/shifted
        u1, s1 = slice(64, 128), slice(0, 64)      # batch1 mirrored

        def pad_memsets(xb, u, s):
            nc.vector.memset(xb[u, 0:PAD], 0.0)
            nc.vector.memset(xb[u, AUD + PAD:PADB], 0.0)
            nc.vector.memset(xb[s, PADB - 1:PADB], 0.0)

        def ph_conv1():
            pad_memsets(xb2s[0], u0, s0)
            pad_memsets(xb2s[1], u1, s1)
            for cc in range(2):
                cv = aps.tile([128, 512], F32, tag="cv")
                nc.tensor.matmul(cv[0:64, :], w1t[:],
                                 a_sbs[0][:, 512 * cc:512 * cc + 512],
                                 start=True, stop=True)
                nc.tensor.matmul(cv[64:128, :], w1t[:],
                                 a_sbs[1][:, 512 * cc:512 * cc + 512],
                                 start=True, stop=True)
                nc.scalar.activation(xb2s[0][u0, PAD + 512 * cc: PAD + 512 * cc + 512],
                                     cv[0:64, :], AF.Gelu, bias=b1c2[0:64])
                nc.scalar.activation(xb2s[1][u1, PAD + 512 * cc: PAD + 512 * cc + 512],
                                     cv[64:128, :], AF.Gelu, bias=b1c2[64:128])
            nc.sync.dma_start(xb2s[0][s0, 0:PADB - 1], xb2s[0][u0, 1:PADB])
            nc.sync.dma_start(xb2s[1][s1, 0:PADB - 1], xb2s[1][u1, 1:PADB])

        x2p = ap.tile([128, AUD], F32, tag="x2p")
        stats = ap.tile([128, 4], F32, tag="stats")

        def ph_conv2():
            sq = ap.tile([128, 512], F32, tag="sq")
            for cc in range(2):
                cv = aps.tile([128, 512], F32, tag="cv")
                for q in range(9):
                    nc.tensor.matmul(cv[0:64, :], w2p[:, q, :],
                                     xb2s[0][:, 2 * q + 512 * cc: 2 * q + 512 * cc + 512],
                                     start=(q == 0), stop=(q == 8))
                    nc.tensor.matmul(cv[64:128, :], w2pB[:, q, :],
                                     xb2s[1][:, 2 * q + 512 * cc: 2 * q + 512 * cc + 512],
                                     start=(q == 0), stop=(q == 8))
                nc.vector.tensor_scalar(
                    out=x2p[:, 512 * cc:512 * cc + 512], in0=cv[:],
                    scalar1=b2c2[:], scalar2=0.0, op0=mybir.AluOpType.add,
                    op1=mybir.AluOpType.add, accum_out=stats[:, cc:cc + 1])
                nc.vector.tensor_mul(sq[:], x2p[:, 512 * cc:512 * cc + 512],
                                     x2p[:, 512 * cc:512 * cc + 512])
                nc.vector.reduce_sum(stats[:, 2 + cc:3 + cc], sq[:],
                                     axis=mybir.AxisListType.X)

        def ph_ln():
            tot16 = ap.tile([128, 2], BF16, tag="tot16")
            nc.vector.tensor_add(tot16[:, 0:1], stats[:, 0:1], stats[:, 1:2])
            nc.vector.tensor_add(tot16[:, 1:2], stats[:, 2:3], stats[:, 3:4])
            totp = aps.tile([128, 64], F32, tag="tp")
            nc.tensor.matmul(totp[:, 0:2], ones2[:], tot16[:], start=True, stop=True)

            mu = ap.tile([128, 1], F32, tag="mu")
            msq = ap.tile([128, 1], F32, tag="msq")
            var = ap.tile([128, 1], F32, tag="var")
            sd = ap.tile([128, 1], F32, tag="sd")
            rstd = ap.tile([128, 1], F32, tag="rstd")
            nmr = ap.tile([128, 1], F32, tag="nmr")
            inv_n = 1.0 / (64 * AUD)
            nc.vector.tensor_scalar_mul(mu[:], totp[:, 0:1], inv_n)
            nc.vector.tensor_scalar_mul(msq[:], totp[:, 1:2], inv_n)
            nc.vector.tensor_mul(var[:], mu[:], mu[:])
            nc.vector.tensor_sub(var[:], msq[:], var[:])
            nc.vector.tensor_scalar_add(var[:], var[:], EPS)
            nc.scalar.activation(sd[:], var[:], AF.Ln)
            nc.scalar.activation(rstd[:], sd[:], AF.Exp, scale=-0.5)
            nc.vector.tensor_scalar(out=nmr[:], in0=mu[:], scalar1=rstd[:],
                                    scalar2=-1.0, op0=mybir.AluOpType.mult,
                                    op1=mybir.AluOpType.mult)

            t1p = ap.tile([128, AUD], F32, tag="t1p")
            t2p = ap.tile([128, AUD], F32, tag="t2p")
            pad_memsets(xb3s[0], u0, s0)
            pad_memsets(xb3s[1], u1, s1)
            nc.vector.tensor_scalar(out=t1p[:], in0=x2p[:], scalar1=rstd[:],
                                    scalar2=nmr[:], op0=mybir.AluOpType.mult,
                                    op1=mybir.AluOpType.add)
            nc.vector.tensor_mul(t2p[:], t1p[:], lnw2[:])
            nc.vector.tensor_add(xb3s[0][u0, PAD:PAD + AUD], t2p[0:64, :],
                                 lnb2[0:64, :])
            nc.vector.tensor_add(xb3s[1][u1, PAD:PAD + AUD], t2p[64:128, :],
                                 lnb2[64:128, :])
            nc.sync.dma_start(xb3s[0][s0, 0:PADB - 1], xb3s[0][u0, 1:PADB])
            nc.sync.dma_start(xb3s[1][s1, 0:PADB - 1], xb3s[1][u1, 1:PADB])

        xpair = ap.tile([128, AUD], BF16, tag="xpair")

        def ph_conv3():
            for cc in range(2):
                cv = aps.tile([128, 512], F32, tag="cv")
                for q in range(9):
                    nc.tensor.matmul(cv[0:64, :], w3p[:, q, :],
                                     xb3s[0][:, 2 * q + 512 * cc: 2 * q + 512 * cc + 512],
                                     start=(q == 0), stop=(q == 8))
                    nc.tensor.matmul(cv[64:128, :], w3pB[:, q, :],
                                     xb3s[1][:, 2 * q + 512 * cc: 2 * q + 512 * cc + 512],
                                     start=(q == 0), stop=(q == 8))
                nc.vector.tensor_scalar(
                    out=xpair[:, 512 * cc:512 * cc + 512], in0=cv[:],
                    scalar1=b3c2[:], scalar2=0.0, op0=mybir.AluOpType.add,
                    op1=mybir.AluOpType.add)

        xt2 = pp.tile([128, 8, 128], BF16, tag="xt2")

        def ph_xt():
            for f in range(8):
                pt = aps.tile([128, 128], BF16, tag="pt")
                nc.tensor.transpose(pt[:], xpair[:, 128 * f:128 * f + 128],
                                    ident[:])
                nc.scalar.activation(xt2[:, f, :], pt[:], AF.Copy)

        kp_all, vp_all = [None, None], [None, None]

        def ph_ktv():
            kt2 = pp.tile([128, 3, 128], BF16, tag="kt2")
            for m in range(3):
                ktp = aps.tile([128, 128], F32, tag="pt")
                for aj in range(8):
                    nc.tensor.matmul(ktp[:], wkt[:, aj, 128 * m:128 * m + 128],
                                     xt2[:, aj, :], start=(aj == 0), stop=(aj == 7))
                nc.scalar.activation(kt2[:, m, :], ktp[:], AF.Copy)

            v2p = aps.tile([128, INNER], F32, tag="cv")
            for aj in range(8):
                nc.tensor.matmul(v2p[:], xt2[:, aj, :], wvt[:, aj, :],
                                 start=(aj == 0), stop=(aj == 7))
            v2 = pp.tile([128, INNER], BF16, tag="v2")
            nc.scalar.activation(v2[:], v2p[:], AF.Copy)
            # per-batch j-duplicated copies (rows j twice) via sbuf-sbuf DMA
            vdup = []
            for b in range(BP):
                t = pp.tile([128, INNER], BF16, tag=f"vdup{b}")
                nc.sync.dma_start(t[0:64, :], v2[64 * b:64 * b + 64, :])
                nc.sync.dma_start(t[64:128, :], v2[64 * b:64 * b + 64, :])
                vdup.append(t)

            for b in range(BP):
                kps = []
                for t, (n, p) in enumerate(KP_DEF):
                    kpt = pp.tile([128, 128], BF16, tag=f"kp{t}_{b}")
                    nc.vector.tensor_mul(
                        kpt[:].rearrange("p (a b) -> p a b", a=2),
                        kt2[:, n:n + 1, 64 * b:64 * b + 64]
                        .broadcast_to([128, 2, 64]),
                        kmask[:, t, :].rearrange("p (a b) -> p a b", a=2))
                    kps.append(kpt)
                kp_all[b] = kps

                vps = {}
                for (n, p) in sorted(set((n, p) for n, p, _, _, _ in VP_OPS)):
                    vpt = pp.tile([128, ATW[n] if n == 2 else 128], BF16,
                                  tag=f"vp{n}{p}_{b}", name=f"vp{n}{p}_{b}")
                    nc.gpsimd.memset(vpt[:], 0.0)
                    vps[(n, p)] = vpt
                for si, (n, p, c0, d0, w) in enumerate(VP_OPS):
                    nc.vector.tensor_mul(vps[(n, p)][:, c0:c0 + w],
                                         vdup[b][:, d0:d0 + w],
                                         vm4[:, si, 0:w])
                for (n, p, col, h) in DEN_SETS:
                    half = slice(0, 64) if h % 2 == 0 else slice(64, 128)
                    nc.gpsimd.memset(vps[(n, p)][half, col:col + 1], 1.0)
                vp_all[b] = vps

        # ---- preamble: audio phases interleaved with q prefill; const
        # loads are emitted at points where their ring is otherwise idle ----
        ph_conv1()
        # WAR-gate the big non-critical loads behind early audio compute so
        # their transfers don't starve the startup-critical tiny DMAs (the
        # scheduler otherwise front-loads everything at t~7us and the a_im
        # completion that gates conv1 drains last behind ~6MB of backlog).
        def stall(tag, shape, dt, src, row=0):
            s = cp.tile(list(shape), dt, tag=tag, name=f"stall_{tag}")
            dst = (s[row:row + 1, 0, 0:4] if len(shape) == 3
                   else s[row:row + 1, 0:4])
            nc.vector.tensor_copy(dst, src)
            return s

        stall("wkt", (128, 8, 384), BF16, xb2s[0][0:1, PAD:PAD + 4])
        stall("wvt", (128, 8, INNER), BF16, xb2s[1][64:65, PAD:PAD + 4],
              row=64)
        stall("lnw2", (128, AUD), F32, xb2s[0][0:1, PAD:PAD + 4])
        stall("lnb2", (128, AUD), F32, xb2s[1][64:65, PAD:PAD + 4], row=64)
        # cin2 as a one-shot gated tile (keeps it out of the startup DMA
        # backlog that delays a_im's completion, which gates conv1)
        for j in (2,):
            s = stall(f"cin{j}", (128, 6, TCH), BF16,
                      xb2s[0][0:1, PAD:PAD + 4])
            t = cp.tile([128, 6, TCH], BF16, tag=f"cin{j}", name=f"cinx{j}")
            nc.gpsimd.dma_start(t[:], P["ctxp"][0, j])
            cins[(0, j)] = t
            cin_loaded.add((0, j))
        # burn the remaining fresh cinp ring buffer behind the same gate so
        # later chunk loads can't be hoisted into the startup window
        for k in range(1):
            d = cinp.tile([128, 6, TCH], BF16, tag="cin", name=f"cind{k}")
            nc.vector.tensor_copy(d[0:1, 0, 0:4], xb2s[0][0:1, PAD:PAD + 4])
        emit_q(0, aps, "qpre")
        lnw2 = cload(nc.sync, "lnw2", (128, AUD), F32)
        lnb2 = cload(nc.sync, "lnb2", (128, AUD), F32)
        w3p = cload(nc.sync, "w3p", (128, 9, 64), BF16)
        w3pB = cload(nc.sync, "w3pB", (128, 9, 64), BF16)
        b3c2 = cload(nc.sync, "b3c2", (128, 1), F32)
        ident = cload(nc.sync, "ident", (128, 128), BF16)
        load_cin(3)
        ph_conv2()
        stall("woutA", (128, 3, CTX), BF16, x2p[0:1, 0:4])
        stall("kmask", (128, 6, 128), BF16, x2p[0:1, 0:4])
        stall("vm4", (128, 9, 80), BF16, x2p[0:1, 0:4])
        stall("exp8a", (68, 3, 128), BF16, x2p[0:1, 0:4])
        stall("exp8b", (68, 3, 128), BF16, x2p[0:1, 0:4])
        load_cin(4)
        emit_q(1, aps, "qpre")
        ph_ln()
        # ACT idles during the LN tail / conv3: load k/v weights now
        wkt = cload(nc.scalar, "wkt", (128, 8, 384), BF16,
                    P["wkt"][:].rearrange("(n p) e -> p n e", p=128))
        wvt = cload(nc.scalar, "wvt", (128, 8, INNER), BF16,
                    P["wvt"][:].rearrange("(n p) e -> p n e", p=128))
        emit_q(2, aps, "qpre")
        emit_q(3, aps, "qpre")
        ph_conv3()
        emit_q(4, aps, "qpre")
        ph_xt()
        kmask = cload(nc.scalar, "kmask", (128, 6, 128), BF16)
        vm4 = cload(nc.scalar, "vm4", (128, 9, 80), BF16)
        exp8a = cload(nc.scalar, "exp8a", (68, 3, 128), BF16)
        exp8b = cload(nc.scalar, "exp8b", (68, 3, 128), BF16)
        woutA = cload(nc.scalar, "woutA", (128, 3, CTX), BF16,
                      P["woutA"][:].rearrange("(n p) c -> p n c", p=128))
        ph_ktv()

        aps.release()

        # ---- main attention loop ----
        mps = tc.alloc_tile_pool(name="mps", bufs=2, space="PSUM")

        pending_out = None

        def emit_out(job):
            ob, oc, oat = job
            for tt in range(4):
                of = ofp.tile([128, CTX], BF16, tag="of")
                for ci, (c0, cw) in enumerate(((0, 384), (384, 384))):
                    op = mps.tile([128, 512], F32, tag="ob")
                    for n in range(3):
                        rows = ATW[n]
                        nc.tensor.matmul(
                            op[:, 0:cw],
                            oat[0:rows, n, 128 * tt:128 * tt + 128],
                            woutA[0:rows, n, c0:c0 + cw],
                            start=(n == 0), stop=(n == 2))
                    if ci == 0:
                        nc.scalar.activation(of[:, c0:c0 + cw], op[:, 0:cw],
                                             AF.Copy)
                    else:
                        nc.vector.tensor_copy(of[:, c0:c0 + cw], op[:, 0:cw])
                nc.sync.dma_start(
                    out_e[ob, TCH * oc + 128 * tt: TCH * oc + 128 * tt + 128, :],
                    of[:])

        chunks = [(bb, cc2) for bb in range(BP) for cc2 in range(NCH)]
        NC_ALL = len(chunks)
        next_q01 = 5

        for i, (b, c) in enumerate(chunks):
            kps = kp_all[b]
            vps = vp_all[b]
            for la in (3, 4):
                if i + la < NC_ALL and chunks[i + la] not in cin_loaded:
                    b3, c3 = chunks[i + la]
                    t = cinp.tile([128, 6, TCH], BF16, tag="cin")
                    nc.gpsimd.dma_start(t[:], P["ctxp"][b3, c3])
                    cins[chunks[i + la]] = t
                    cin_loaded.add(chunks[i + la])

            qt = qts.pop(i)

            es = []
            for p in range(4):
                sp = mps.tile([128, TCH], F32, tag="sp")
                plan = SIM_PLAN[p]
                for ii, (kpi, qch) in enumerate(plan):
                    nc.tensor.matmul(sp[:], kps[kpi][:], qt[:, qch, :],
                                     start=(ii == 0), stop=(ii == len(plan) - 1))
                e = esp.tile([128, TCH], BF16, tag="es")
                nc.scalar.activation(e[:], sp[:], AF.Exp, scale=SCALE)
                es.append(e)

            # at tiles T0, T1 (denominators embedded at rows 96:104)
            atps = []
            for n in (0, 1):
                a = mps.tile([128, TCH], F32, tag="at")
                prs = AV_PLAN[n]
                for ii, p in enumerate(prs):
                    nc.tensor.matmul(a[0:ATW[n], :], vps[(n, p)][:, 0:ATW[n]],
                                     es[p][:], start=(ii == 0),
                                     stop=(ii == len(prs) - 1))
                atps.append(a)

            # q for upcoming chunks fills the PE while exp/ln run on ACT
            if next_q01 < NC_ALL and next_q01 <= i + 2:
                emit_q(next_q01, mps, "qp")
                next_q01 += 1

            at_sb = mp.tile([128, 3, TCH], BF16, tag="at_sb")

            lnd = mp.tile([68, TCH], F32, tag="lnd")
            lndb = mp.tile([68, TCH], F32, tag="lndb")
            recs = {"a": mp.tile([68, TCH], BF16, tag="recA", name="recA"),
                    "b": mp.tile([68, TCH], BF16, tag="recB", name="recB")}
            e8s = {"a": exp8a, "b": exp8b}
            nc.scalar.activation(lnd[64:68, :], atps[0][64:68, :], AF.Ln)
            nc.scalar.activation(recs["a"][64:68, :], lnd[64:68, :], AF.Exp,
                                 scale=-1.0)
            nc.scalar.activation(lndb[64:68, :], atps[1][64:68, :], AF.Ln)
            nc.scalar.activation(recs["b"][64:68, :], lndb[64:68, :], AF.Exp,
                                 scale=-1.0)

            def normalize(n, at_ps):
                brp = mps.tile([128, TCH], F32, tag="ob")
                srcs = NORM_SRC[n]
                for si, sk in enumerate(srcs):
                    nc.tensor.matmul(brp[0:BW[n], :],
                                     e8s[sk][64:68, n, 0:BW[n]],
                                     recs[sk][64:68, :],
                                     start=(si == 0), stop=(si == len(srcs) - 1))
                brs = mp.tile([128, TCH], BF16, tag="brs")
                nc.vector.tensor_copy(brs[0:BW[n], :], brp[0:BW[n], :])
                nc.vector.tensor_mul(at_sb[0:BW[n], n, :],
                                     at_ps[0:BW[n], :], brs[0:BW[n], :])

            normalize(0, atps[0])
            a2 = mps.tile([128, TCH], F32, tag="sp")
            nc.tensor.matmul(a2[0:ATW[2], :], vps[(2, 3)][:, 0:ATW[2]],
                             es[3][:], start=True, stop=True)
            normalize(1, atps[1])
            normalize(2, a2)
            # bias ones-row (T2 row 64) — after normalize(2)'s mul zeroed it
            nc.gpsimd.memset(at_sb[64:65, 2, :], 1.0)

            if pending_out is not None:
                emit_out(pending_out)
            pending_out = (b, c, at_sb)

        emit_out(pending_out)

        mps.release()
        ap.release()
        ofp.release()
        mp.release()
        qtp.release()
        esp.release()
        cinp.release()
        pp.release()
        cp.release()

    split_waits(nc)
    return nc


def split_waits(nc, max_waits=1):
    """neuronxcc walrus accepts at most one attached sync wait per
    instruction; hoist extras onto standalone event-semaphore waits."""
    n_new = 0
    for f in nc.m.functions:
        for blk in f.blocks:
            new = []
            changed = False
            for inst in blk.instructions:
                si = inst.sync_info
                ow = list(si.on_wait) if (si is not None and si.on_wait) else []
                if len(ow) > max_waits:
                    for w in ow[:-max_waits]:
                        ev = mybir.InstEventSemaphore(
                            name=f"I-waitsplit-{n_new}", ins=[], outs=[])
                        ev.engine = inst.engine
                        ev.sync_info = mybir.SyncInfo(on_wait=[w], on_update=[])
                        nc.register_instruction(ev)
                        new.append(ev)
                        n_new += 1
                    inst.sync_info = mybir.SyncInfo(
                        on_wait=ow[-max_waits:], on_update=list(si.on_update))
                    changed = True
                new.append(inst)
            if changed:
                blk.instructions = new


_GRAPH = None


def _prep_in_maps(inputs):
    inputs = {k: np.asarray(v, dtype=np.float32) for k, v in inputs.items()}
    consts = _build_host_consts(inputs)
    ctx = np.asarray(inputs["context"])           # [16, 4096, 768] f32
    audio = np.asarray(inputs["audio_context"])   # [16, 1, 1024] f32

    # pack ctx so each [128, 6, 512] chunk tile is contiguous in DRAM
    ctx16 = ctx.astype(BF)
    ctxp = np.ascontiguousarray(
        ctx16.transpose(0, 2, 1).reshape(B, 6, 128, NCH, TCH)
        .transpose(0, 3, 2, 1, 4))                # [B, NCH, 128, 6, TCH]
    apad = np.zeros((B, AUD + 2 * PAD), np.float32)
    apad[:, PAD:PAD + AUD] = audio[:, 0, :]
    a_im = np.empty((B, KS, AUD), np.float32)
    for k in range(KS):
        a_im[:, k, :] = apad[:, k:k + AUD]
    a_im = a_im.astype(BF)

    in_maps = []
    for core in range(NCORES):
        m = dict(consts)
        s = slice(core * BP, (core + 1) * BP)
        m["ctxp"] = ctxp[s]
        m["a_im"] = a_im[s]
        in_maps.append(m)
    return in_maps


def kernel(**inputs):
    global _GRAPH
    if _GRAPH is None:
        _GRAPH = _build_graph()
    nc = _GRAPH

    in_maps = _prep_in_maps(inputs)
    res = run_bass_kernel_spmd(nc, in_maps, list(range(NCORES)))
    out = np.concatenate([res.results[i]["out"] for i in range(NCORES)], axis=0)
    return out.astype(np.float32)


# revision 70
# speedup vs baseline: 1.0205x; 1.0205x over previous
"""Trainium2 Bass kernel for nn_Adapter (audio conv encoder + cross-attention).

Data-parallel over batch: 16 batches / 8 NeuronCores = 2 per core, no
collectives. All heavy matmuls run in bf16 (1 cycle/row on the PE array);
PSUM accumulation is fp32 throughout. Output is written bf16 and upcast on
host.

v2 vs v1:
 - audio encoder processes both batches concurrently (column-tiled convs,
   batch-packed transposes / k / v projections), entirely up front.
 - q-projection's half-wide third m-tile is computed for two token chunks
   at once via column tiling.
 - startup DMAs are spread across the sync/scalar HWDGE rings and the
   gpsimd SWDGE ring in dependency order; context chunks are host-packed
   so each chunk tile is one contiguous DMA.
 - output tensor is bf16.
"""
import sys
sys.path.insert(0, "/opt/trn_rl_repo")

import numpy as np
import ml_dtypes

import concourse.bass as bass
import concourse.mybir as mybir
import concourse.tile as tile
from concourse.bass_utils import run_bass_kernel_spmd

F32 = mybir.dt.float32
BF16 = mybir.dt.bfloat16
AF = mybir.ActivationFunctionType
BF = ml_dtypes.bfloat16

NCORES = 8
B, N, CTX = 16, 4096, 768
BP = B // NCORES            # batches per core
H, D, INNER = 8, 40, 320    # heads, dim_head, inner
AUD = 1024                  # audio feature length
KS, PAD = 17, 8
EPS = 1e-5
SCALE = D ** -0.5
TCH = 512                   # token chunk
NCH = N // TCH              # chunks per batch
PADB = AUD + 2 * PAD

# pair -> sim matmul plan: (kp_tile_index, qt_chunk)
SIM_PLAN = [
    [(0, 0)],           # pair 0 (h0,h1): KP01 x qt_ch0
    [(1, 0), (2, 1)],   # pair 1 (h2,h3): KP23a x ch0 + KP23b x ch1
    [(3, 1)],           # pair 2 (h4,h5): KP45 x ch1
    [(4, 1), (5, 2)],   # pair 3 (h6,h7): KP67a x ch1 + KP67b x ch2
]
KP_DEF = [(0, 0), (0, 1), (1, 1), (1, 2), (1, 3), (2, 3)]  # tile -> (chunk, pair)
ME = [128, 128, 64]                                         # e-chunk sizes

# attention-value (at) tile row layout: e-dims distributed over 3 tiles.
# The 8 softmax denominators sit at rows 64:68 (a 64-aligned base, required
# by the partition-start rules) of T0 (heads 0-3) and T1 (heads 4-7); the
# out-proj bias ones-row is T2 row 64.  Each at tile is produced by
# contracting only the es pairs it touches: T0: p0,p1  T1: p1,p2,p3  T2: p3
# (6 matmuls).  Normalization broadcasts come from two reciprocal tiles
# (recA: heads 0-3, recB: heads 4-7) -> 1+2+1 = 4 broadcast matmuls.
AV_PLAN = {0: [0, 1], 1: [1, 2, 3], 2: [3]}
ATW = [128, 128, 73]            # at-tile matmul/emit widths
BW = [128, 128, 73]             # normalize widths
NORM_SRC = {0: ["a"], 1: ["a", "b"], 2: ["b"]}   # which rec tiles per at tile
# vp build ops: (tile n, pair p, dst col0, src e-dim d0, width)
VP_OPS = [
    (0, 0, 0, 0, 64),
    (0, 0, 68, 64, 16),
    (0, 1, 84, 80, 44),
    (1, 1, 0, 124, 36),
    (1, 2, 36, 160, 28),
    (1, 2, 68, 188, 52),
    (1, 3, 120, 240, 8),
    (2, 3, 0, 248, 64),
    (2, 3, 65, 312, 8),
]
# denominator ones columns: (tile n, pair p, col, head)
DEN_SETS = [
    (0, 0, 64, 0), (0, 0, 65, 1), (0, 1, 66, 2), (0, 1, 67, 3),
    (1, 2, 64, 4), (1, 2, 65, 5), (1, 3, 66, 6), (1, 3, 67, 7),
]


def _head_of(e):
    return e // D


def _at_row_dim(n, r):
    """at-tile row -> e-dim, or None for denominator/bias rows."""
    if n == 0:
        if 64 <= r < 68:
            return None
        return r if r < 64 else r - 4
    if n == 1:
        if 64 <= r < 68:
            return None
        return 124 + r if r < 64 else 120 + r
    if r < 64:
        return 248 + r
    return None if r == 64 else 247 + r


def _build_host_consts(inputs):
    c = {}
    w1, b1 = inputs["w1"], inputs["b1"]
    w2, b2 = inputs["w2"], inputs["b2"]
    w3, b3 = inputs["w3"], inputs["b3"]
    c["w1t"] = np.ascontiguousarray(w1[:, 0, :].T).astype(BF)             # [17, 64]

    def pack_pairs(w):  # [64co, 64ci, 17] -> [128, 9, 64co]
        wp = np.zeros((128, 9, 64), np.float32)
        wt = w.transpose(1, 2, 0)  # [ci, k, co]
        for q in range(9):
            wp[0:64, q, :] = wt[:, 2 * q, :]
            if 2 * q + 1 < KS:
                wp[64:128, q, :] = wt[:, 2 * q + 1, :]
        return wp.astype(BF)

    c["w2p"] = pack_pairs(w2)
    c["w2pB"] = np.concatenate([c["w2p"][64:128], c["w2p"][0:64]], axis=0)
    c["w3p"] = pack_pairs(w3)
    c["w3pB"] = np.concatenate([c["w3p"][64:128], c["w3p"][0:64]], axis=0)
    c["b1c2"] = np.tile(np.asarray(b1).reshape(64, 1), (2, 1)).astype(np.float32)
    c["b2c2"] = np.tile(np.asarray(b2).reshape(64, 1), (2, 1)).astype(np.float32)
    c["b3c2"] = np.tile(np.asarray(b3).reshape(64, 1), (2, 1)).astype(np.float32)
    c["lnw2"] = np.tile(np.asarray(inputs["ln_w"]), (2, 1)).astype(np.float32)
    c["lnb2"] = np.tile(np.asarray(inputs["ln_b"]), (2, 1)).astype(np.float32)

    # cols 320:384 stay zero: the unused array half multiplies by zeros,
    # which keeps PE switching power down (a packed non-zero duplicate was
    # measured to downclock the whole chip to 2.0 GHz)
    wqt = np.zeros((CTX, 384), np.float32)
    wqt[:, :INNER] = np.asarray(inputs["wq"]).T
    c["wqt"] = wqt.astype(BF)
    wkt = np.zeros((AUD, 384), np.float32)
    wkt[:, :INNER] = np.asarray(inputs["wk"]).T
    c["wkt"] = wkt.astype(BF)
    c["wvt"] = np.ascontiguousarray(np.asarray(inputs["wv"]).T).astype(BF)
    woutT = np.asarray(inputs["w_out"]).T          # [320 e, 768]
    wout = np.zeros((384, CTX), np.float32)
    for n in range(3):
        for r in range(ATW[n]):
            d = _at_row_dim(n, r)
            if d is not None:
                wout[128 * n + r] = woutT[d]
    wout[128 * 2 + 64] = np.asarray(inputs["b_out"])
    c["woutA"] = wout.astype(BF)

    km = np.zeros((128, 6, 128), np.float32)
    for t, (n, p) in enumerate(KP_DEF):
        for r in range(ME[n]):
            h = _head_of(128 * n + r)
            if h == 2 * p:
                km[r, t, 0:64] = 1.0
            elif h == 2 * p + 1:
                km[r, t, 64:128] = 1.0
    c["kmask"] = km.astype(BF)

    vm = np.zeros((128, 9, 80), np.float32)
    for si, (n, p, c0, d0, w) in enumerate(VP_OPS):
        for cc in range(w):
            h = _head_of(d0 + cc)
            vm[64 * (h % 2):64 * (h % 2) + 64, si, cc] = 1.0
    c["vm4"] = vm.astype(BF)

    e8a = np.zeros((68, 3, 128), np.float32)
    e8b = np.zeros((68, 3, 128), np.float32)
    for n in range(3):
        for r in range(ATW[n]):
            d = _at_row_dim(n, r)
            if d is not None:
                h = _head_of(d)
                if h < 4:
                    e8a[64 + h, n, r] = 1.0
                else:
                    e8b[60 + h, n, r] = 1.0
    c["exp8a"] = e8a.astype(BF)
    c["exp8b"] = e8b.astype(BF)

    c["ident"] = np.eye(128, dtype=np.float32).astype(BF)
    return c


def _build_graph():
    nc = bass.Bass()
    P = {}

    def inp(name, shape, dt):
        P[name] = nc.declare_dram_parameter(name, list(shape), dt, isOutput=False)

    inp("ctxp", (BP, NCH, 128, 6, TCH), BF16)
    inp("a_im", (BP, KS, AUD), BF16)
    inp("w1t", (KS, 64), BF16)
    inp("w2p", (128, 9, 64), BF16)
    inp("w2pB", (128, 9, 64), BF16)
    inp("w3p", (128, 9, 64), BF16)
    inp("w3pB", (128, 9, 64), BF16)
    inp("b1c2", (128, 1), F32)
    inp("b2c2", (128, 1), F32)
    inp("b3c2", (128, 1), F32)
    inp("lnw2", (128, AUD), F32)
    inp("lnb2", (128, AUD), F32)
    inp("wqt", (CTX, 384), BF16)
    inp("wkt", (AUD, 384), BF16)
    inp("wvt", (AUD, INNER), BF16)
    inp("woutA", (384, CTX), BF16)
    inp("kmask", (128, 6, 128), BF16)
    inp("vm4", (128, 9, 80), BF16)
    inp("exp8a", (68, 3, 128), BF16)
    inp("exp8b", (68, 3, 128), BF16)
    inp("ident", (128, 128), BF16)
    out_e = nc.declare_dram_parameter("out", [BP, N, CTX], BF16, isOutput=True)

    with tile.TileContext(nc) as tc:
        cp = tc.alloc_tile_pool(name="const", bufs=1)
        pp = tc.alloc_tile_pool(name="persist", bufs=1)
        cinp = tc.alloc_tile_pool(name="cinp", bufs=5)
        esp = tc.alloc_tile_pool(name="esp", bufs=6)
        qtp = tc.alloc_tile_pool(name="qtp", bufs=5)
        mp = tc.alloc_tile_pool(name="mp", bufs=2)
        ofp = tc.alloc_tile_pool(name="ofp", bufs=4)
        ap = tc.alloc_tile_pool(name="audio", bufs=1)
        aps = tc.alloc_tile_pool(name="aps", bufs=2, space="PSUM")

        def cload(eng, name, shape, dt, ap_src=None):
            t = cp.tile(list(shape), dt, tag=name)
            eng.dma_start(t[:], ap_src if ap_src is not None else P[name][:])
            return t

        # ---- startup DMAs ----
        # HWDGE dma issues occupy the issuing engine's stream, so the
        # scalar (ACT) ring must stay nearly empty before the first gelu;
        # later consts are emitted mid-preamble at points where that
        # engine idles anyway.
        # warm the ACT function tables before anything depends on ACT
        dummy = cp.tile([1, 4], F32, tag="dummy")
        nc.vector.memset(dummy[:], 1.0)
        nc.scalar.activation(dummy[:], dummy[:], AF.Gelu)
        nc.scalar.activation(dummy[:], dummy[:], AF.Ln)
        nc.scalar.activation(dummy[:], dummy[:], AF.Exp)

        w1t = cload(nc.sync, "w1t", (KS, 64), BF16)
        a_sbs = {}
        for b in range(BP):
            t = ap.tile([KS, AUD], BF16, tag=f"a_im{b}")
            nc.sync.dma_start(t[:], P["a_im"][b])
            a_sbs[b] = t
        b1c2 = cload(nc.sync, "b1c2", (128, 1), F32)
        w2p = cload(nc.sync, "w2p", (128, 9, 64), BF16)
        w2pB = cload(nc.sync, "w2pB", (128, 9, 64), BF16)
        b2c2 = cload(nc.sync, "b2c2", (128, 1), F32)

        # scalar ring: only the q weights early (first ACT op is gelu)
        wqt = cload(nc.scalar, "wqt", (128, 6, 384), BF16,
                    P["wqt"][:].rearrange("(n p) e -> p n e", p=128))

        # gpsimd ring: first two context chunks (more are loaded mid-preamble)
        cins = {}
        cin_loaded = set()

        def load_cin(j):
            t = cinp.tile([128, 6, TCH], BF16, tag="cin", name=f"cin{j}")
            nc.gpsimd.dma_start(t[:], P["ctxp"][j // NCH, j % NCH])
            cins[(j // NCH, j % NCH)] = t
            cin_loaded.add((j // NCH, j % NCH))

        load_cin(0)
        load_cin(1)
        load_cin(2)

        # declared here, loaded mid-preamble (see audio phase emission)
        w3p = w3pB = b3c2 = ident = lnw2 = lnb2 = None
        wkt = wvt = woutA = kmask = vm4 = exp8a = exp8b = None

        # block-diagonal ones for the pair layernorm reduction
        ones2 = cp.tile([128, 128], BF16, tag="ones2")
        nc.vector.memset(ones2[:], 0.0)
        nc.vector.memset(ones2[0:64, 0:64], 1.0)
        nc.vector.memset(ones2[64:128, 64:128], 1.0)

        # ---- q emission helper ----
        qts = {}

        def emit_q(i, psum_pool, psum_tag):
            cin = cins.pop((i // NCH, i % NCH))
            qt = qtp.tile([128, 3, TCH], BF16, tag="qt")
            for m in range(3):
                qp = psum_pool.tile([128, TCH], F32, tag=psum_tag)
                for n6 in range(6):
                    nc.tensor.matmul(qp[:], wqt[:, n6, 128 * m:128 * m + 128],
                                     cin[:, n6, :], start=(n6 == 0), stop=(n6 == 5))
                nc.vector.tensor_copy(qt[:, m, :], qp[:])
            qts[i] = qt
            return qt

        # ---- audio encoder, both batches at once ----
        # batch 0 layout: unshifted rows 0:64, shifted rows 64:128
        # batch 1 layout: mirrored (weights w2pB/w3pB are row-swapped)
        xb2s, xb3s = {}, {}
        for b in range(BP):
            xb2s[b] = ap.tile([128, PADB], BF16, tag=f"xb2{b}",
                              name=f"xb2_{b}")
            xb3s[b] = ap.tile([128, PADB], BF16, tag=f"xb3{b}",
                              name=f"xb3_{b}")
        u0, s0 = slice(0, 64), slice(64, 128)      # batch0 unshifted/shifted
        u1, s1 = slice(64, 128), slice(0, 64)      # batch1 mirrored

        def pad_memsets(xb, u, s):
            nc.vector.memset(xb[u, 0:PAD], 0.0)
            nc.vector.memset(xb[u, AUD + PAD:PADB], 0.0)
            nc.vector.memset(xb[s, PADB - 1:PADB], 0.0)

        def ph_conv1():
            pad_memsets(xb2s[0], u0, s0)
            pad_memsets(xb2s[1], u1, s1)
            for cc in range(2):
                cv = aps.tile([128, 512], F32, tag="cv")
                nc.tensor.matmul(cv[0:64, :], w1t[:],
                                 a_sbs[0][:, 512 * cc:512 * cc + 512],
                                 start=True, stop=True)
                nc.tensor.matmul(cv[64:128, :], w1t[:],
                                 a_sbs[1][:, 512 * cc:512 * cc + 512],
                                 start=True, stop=True)
                nc.scalar.activation(xb2s[0][u0, PAD + 512 * cc: PAD + 512 * cc + 512],
                                     cv[0:64, :], AF.Gelu, bias=b1c2[0:64])
                nc.scalar.activation(xb2s[1][u1, PAD + 512 * cc: PAD + 512 * cc + 512],
                                     cv[64:128, :], AF.Gelu, bias=b1c2[64:128])
            nc.sync.dma_start(xb2s[0][s0, 0:PADB - 1], xb2s[0][u0, 1:PADB])
            nc.sync.dma_start(xb2s[1][s1, 0:PADB - 1], xb2s[1][u1, 1:PADB])

        x2p = ap.tile([128, AUD], F32, tag="x2p")
        stats = ap.tile([128, 4], F32, tag="stats")

        def ph_conv2():
            sq = ap.tile([128, 512], F32, tag="sq")
            for cc in range(2):
                cv = aps.tile([128, 512], F32, tag="cv")
                for q in range(9):
                    nc.tensor.matmul(cv[0:64, :], w2p[:, q, :],
                                     xb2s[0][:, 2 * q + 512 * cc: 2 * q + 512 * cc + 512],
                                     start=(q == 0), stop=(q == 8))
                    nc.tensor.matmul(cv[64:128, :], w2pB[:, q, :],
                                     xb2s[1][:, 2 * q + 512 * cc: 2 * q + 512 * cc + 512],
                                     start=(q == 0), stop=(q == 8))
                nc.vector.tensor_scalar(
                    out=x2p[:, 512 * cc:512 * cc + 512], in0=cv[:],
                    scalar1=b2c2[:], scalar2=0.0, op0=mybir.AluOpType.add,
                    op1=mybir.AluOpType.add, accum_out=stats[:, cc:cc + 1])
                nc.vector.tensor_mul(sq[:], x2p[:, 512 * cc:512 * cc + 512],
                                     x2p[:, 512 * cc:512 * cc + 512])
                nc.vector.reduce_sum(stats[:, 2 + cc:3 + cc], sq[:],
                                     axis=mybir.AxisListType.X)

        def ph_ln():
            tot16 = ap.tile([128, 2], BF16, tag="tot16")
            nc.vector.tensor_add(tot16[:, 0:1], stats[:, 0:1], stats[:, 1:2])
            nc.vector.tensor_add(tot16[:, 1:2], stats[:, 2:3], stats[:, 3:4])
            totp = aps.tile([128, 64], F32, tag="tp")
            nc.tensor.matmul(totp[:, 0:2], ones2[:], tot16[:], start=True, stop=True)

            mu = ap.tile([128, 1], F32, tag="mu")
            msq = ap.tile([128, 1], F32, tag="msq")
            var = ap.tile([128, 1], F32, tag="var")
            sd = ap.tile([128, 1], F32, tag="sd")
            rstd = ap.tile([128, 1], F32, tag="rstd")
            nmr = ap.tile([128, 1], F32, tag="nmr")
            inv_n = 1.0 / (64 * AUD)
            nc.vector.tensor_scalar_mul(mu[:], totp[:, 0:1], inv_n)
            nc.vector.tensor_scalar_mul(msq[:], totp[:, 1:2], inv_n)
            nc.vector.tensor_mul(var[:], mu[:], mu[:])
            nc.vector.tensor_sub(var[:], msq[:], var[:])
            nc.vector.tensor_scalar_add(var[:], var[:], EPS)
            nc.scalar.activation(sd[:], var[:], AF.Ln)
            nc.scalar.activation(rstd[:], sd[:], AF.Exp, scale=-0.5)
            nc.vector.tensor_scalar(out=nmr[:], in0=mu[:], scalar1=rstd[:],
                                    scalar2=-1.0, op0=mybir.AluOpType.mult,
                                    op1=mybir.AluOpType.mult)

            t1p = ap.tile([128, AUD], F32, tag="t1p")
            t2p = ap.tile([128, AUD], F32, tag="t2p")
            pad_memsets(xb3s[0], u0, s0)
            pad_memsets(xb3s[1], u1, s1)
            nc.vector.tensor_scalar(out=t1p[:], in0=x2p[:], scalar1=rstd[:],
                                    scalar2=nmr[:], op0=mybir.AluOpType.mult,
                                    op1=mybir.AluOpType.add)
            nc.vector.tensor_mul(t2p[:], t1p[:], lnw2[:])
            nc.vector.tensor_add(xb3s[0][u0, PAD:PAD + AUD], t2p[0:64, :],
                                 lnb2[0:64, :])
            nc.vector.tensor_add(xb3s[1][u1, PAD:PAD + AUD], t2p[64:128, :],
                                 lnb2[64:128, :])
            nc.sync.dma_start(xb3s[0][s0, 0:PADB - 1], xb3s[0][u0, 1:PADB])
            nc.sync.dma_start(xb3s[1][s1, 0:PADB - 1], xb3s[1][u1, 1:PADB])

        xpair = ap.tile([128, AUD], BF16, tag="xpair")

        def ph_conv3():
            for cc in range(2):
                cv = aps.tile([128, 512], F32, tag="cv")
                for q in range(9):
                    nc.tensor.matmul(cv[0:64, :], w3p[:, q, :],
                                     xb3s[0][:, 2 * q + 512 * cc: 2 * q + 512 * cc + 512],
                                     start=(q == 0), stop=(q == 8))
                    nc.tensor.matmul(cv[64:128, :], w3pB[:, q, :],
                                     xb3s[1][:, 2 * q + 512 * cc: 2 * q + 512 * cc + 512],
                                     start=(q == 0), stop=(q == 8))
                nc.vector.tensor_scalar(
                    out=xpair[:, 512 * cc:512 * cc + 512], in0=cv[:],
                    scalar1=b3c2[:], scalar2=0.0, op0=mybir.AluOpType.add,
                    op1=mybir.AluOpType.add)

        xt2 = pp.tile([128, 8, 128], BF16, tag="xt2")

        def ph_xt():
            for f in range(8):
                pt = aps.tile([128, 128], BF16, tag="pt")
                nc.tensor.transpose(pt[:], xpair[:, 128 * f:128 * f + 128],
                                    ident[:])
                nc.scalar.activation(xt2[:, f, :], pt[:], AF.Copy)

        kp_all, vp_all = [None, None], [None, None]

        def ph_ktv():
            kt2 = pp.tile([128, 3, 128], BF16, tag="kt2")
            for m in range(3):
                ktp = aps.tile([128, 128], F32, tag="pt")
                for aj in range(8):
                    nc.tensor.matmul(ktp[:], wkt[:, aj, 128 * m:128 * m + 128],
                                     xt2[:, aj, :], start=(aj == 0), stop=(aj == 7))
                nc.scalar.activation(kt2[:, m, :], ktp[:], AF.Copy)

            v2p = aps.tile([128, INNER], F32, tag="cv")
            for aj in range(8):
                nc.tensor.matmul(v2p[:], xt2[:, aj, :], wvt[:, aj, :],
                                 start=(aj == 0), stop=(aj == 7))
            v2 = pp.tile([128, INNER], BF16, tag="v2")
            nc.scalar.activation(v2[:], v2p[:], AF.Copy)
            # per-batch j-duplicated copies (rows j twice) via sbuf-sbuf DMA
            vdup = []
            for b in range(BP):
                t = pp.tile([128, INNER], BF16, tag=f"vdup{b}")
                nc.sync.dma_start(t[0:64, :], v2[64 * b:64 * b + 64, :])
                nc.sync.dma_start(t[64:128, :], v2[64 * b:64 * b + 64, :])
                vdup.append(t)

            for b in range(BP):
                kps = []
                for t, (n, p) in enumerate(KP_DEF):
                    kpt = pp.tile([128, 128], BF16, tag=f"kp{t}_{b}")
                    nc.vector.tensor_mul(
                        kpt[:].rearrange("p (a b) -> p a b", a=2),
                        kt2[:, n:n + 1, 64 * b:64 * b + 64]
                        .broadcast_to([128, 2, 64]),
                        kmask[:, t, :].rearrange("p (a b) -> p a b", a=2))
                    kps.append(kpt)
                kp_all[b] = kps

                vps = {}
                for (n, p) in sorted(set((n, p) for n, p, _, _, _ in VP_OPS)):
                    vpt = pp.tile([128, ATW[n] if n == 2 else 128], BF16,
                                  tag=f"vp{n}{p}_{b}", name=f"vp{n}{p}_{b}")
                    nc.gpsimd.memset(vpt[:], 0.0)
                    vps[(n, p)] = vpt
                for si, (n, p, c0, d0, w) in enumerate(VP_OPS):
                    nc.vector.tensor_mul(vps[(n, p)][:, c0:c0 + w],
                                         vdup[b][:, d0:d0 + w],
                                         vm4[:, si, 0:w])
                for (n, p, col, h) in DEN_SETS:
                    half = slice(0, 64) if h % 2 == 0 else slice(64, 128)
                    nc.gpsimd.memset(vps[(n, p)][half, col:col + 1], 1.0)
                vp_all[b] = vps

        # ---- preamble: audio phases interleaved with q prefill; const
        # loads are emitted at points where their ring is otherwise idle ----
        ph_conv1()
        emit_q(0, aps, "qpre")
        lnw2 = cload(nc.sync, "lnw2", (128, AUD), F32)
        lnb2 = cload(nc.sync, "lnb2", (128, AUD), F32)
        w3p = cload(nc.sync, "w3p", (128, 9, 64), BF16)
        w3pB = cload(nc.sync, "w3pB", (128, 9, 64), BF16)
        b3c2 = cload(nc.sync, "b3c2", (128, 1), F32)
        ident = cload(nc.sync, "ident", (128, 128), BF16)
        load_cin(3)
        ph_conv2()
        load_cin(4)
        emit_q(1, aps, "qpre")
        ph_ln()
        # ACT idles during the LN tail / conv3: load k/v weights now
        wkt = cload(nc.scalar, "wkt", (128, 8, 384), BF16,
                    P["wkt"][:].rearrange("(n p) e -> p n e", p=128))
        wvt = cload(nc.scalar, "wvt", (128, 8, INNER), BF16,
                    P["wvt"][:].rearrange("(n p) e -> p n e", p=128))
        emit_q(2, aps, "qpre")
        emit_q(3, aps, "qpre")
        ph_conv3()
        emit_q(4, aps, "qpre")
        ph_xt()
        kmask = cload(nc.scalar, "kmask", (128, 6, 128), BF16)
        vm4 = cload(nc.scalar, "vm4", (128, 9, 80), BF16)
        exp8a = cload(nc.scalar, "exp8a", (68, 3, 128), BF16)
        exp8b = cload(nc.scalar, "exp8b", (68, 3, 128), BF16)
        woutA = cload(nc.scalar, "woutA", (128, 3, CTX), BF16,
                      P["woutA"][:].rearrange("(n p) c -> p n c", p=128))
        ph_ktv()

        aps.release()

        # ---- main attention loop ----
        mps = tc.alloc_tile_pool(name="mps", bufs=2, space="PSUM")

        pending_out = None

        def emit_out(job):
            ob, oc, oat = job
            for tt in range(4):
                of = ofp.tile([128, CTX], BF16, tag="of")
                for ci, (c0, cw) in enumerate(((0, 384), (384, 384))):
                    op = mps.tile([128, 512], F32, tag="ob")
                    for n in range(3):
                        rows = ATW[n]
                        nc.tensor.matmul(
                            op[:, 0:cw],
                            oat[0:rows, n, 128 * tt:128 * tt + 128],
                            woutA[0:rows, n, c0:c0 + cw],
                            start=(n == 0), stop=(n == 2))
                    if ci == 0:
                        nc.scalar.activation(of[:, c0:c0 + cw], op[:, 0:cw],
                                             AF.Copy)
                    else:
                        nc.vector.tensor_copy(of[:, c0:c0 + cw], op[:, 0:cw])
                nc.sync.dma_start(
                    out_e[ob, TCH * oc + 128 * tt: TCH * oc + 128 * tt + 128, :],
                    of[:])

        chunks = [(bb, cc2) for bb in range(BP) for cc2 in range(NCH)]
        NC_ALL = len(chunks)
        next_q01 = 5

        for i, (b, c) in enumerate(chunks):
            kps = kp_all[b]
            vps = vp_all[b]
            for la in (3, 4):
                if i + la < NC_ALL and chunks[i + la] not in cin_loaded:
                    b3, c3 = chunks[i + la]
                    t = cinp.tile([128, 6, TCH], BF16, tag="cin")
                    nc.gpsimd.dma_start(t[:], P["ctxp"][b3, c3])
                    cins[chunks[i + la]] = t
                    cin_loaded.add(chunks[i + la])

            qt = qts.pop(i)

            es = []
            for p in range(4):
                sp = mps.tile([128, TCH], F32, tag="sp")
                plan = SIM_PLAN[p]
                for ii, (kpi, qch) in enumerate(plan):
                    nc.tensor.matmul(sp[:], kps[kpi][:], qt[:, qch, :],
                                     start=(ii == 0), stop=(ii == len(plan) - 1))
                e = esp.tile([128, TCH], BF16, tag="es")
                nc.scalar.activation(e[:], sp[:], AF.Exp, scale=SCALE)
                es.append(e)

            # at tiles T0, T1 (denominators embedded at rows 96:104)
            atps = []
            for n in (0, 1):
                a = mps.tile([128, TCH], F32, tag="at")
                prs = AV_PLAN[n]
                for ii, p in enumerate(prs):
                    nc.tensor.matmul(a[0:ATW[n], :], vps[(n, p)][:, 0:ATW[n]],
                                     es[p][:], start=(ii == 0),
                                     stop=(ii == len(prs) - 1))
                atps.append(a)

            # q for upcoming chunks fills the PE while exp/ln run on ACT
            if next_q01 < NC_ALL and next_q01 <= i + 2:
                emit_q(next_q01, mps, "qp")
                next_q01 += 1

            at_sb = mp.tile([128, 3, TCH], BF16, tag="at_sb")

            lnd = mp.tile([68, TCH], F32, tag="lnd")
            lndb = mp.tile([68, TCH], F32, tag="lndb")
            recs = {"a": mp.tile([68, TCH], BF16, tag="recA", name="recA"),
                    "b": mp.tile([68, TCH], BF16, tag="recB", name="recB")}
            e8s = {"a": exp8a, "b": exp8b}
            nc.scalar.activation(lnd[64:68, :], atps[0][64:68, :], AF.Ln)
            nc.scalar.activation(recs["a"][64:68, :], lnd[64:68, :], AF.Exp,
                                 scale=-1.0)
            nc.scalar.activation(lndb[64:68, :], atps[1][64:68, :], AF.Ln)
            nc.scalar.activation(recs["b"][64:68, :], lndb[64:68, :], AF.Exp,
                                 scale=-1.0)

            def normalize(n, at_ps):
                brp = mps.tile([128, TCH], F32, tag="ob")
                srcs = NORM_SRC[n]
                for si, sk in enumerate(srcs):
                    nc.tensor.matmul(brp[0:BW[n], :],
                                     e8s[sk][64:68, n, 0:BW[n]],
                                     recs[sk][64:68, :],
                                     start=(si == 0), stop=(si == len(srcs) - 1))
                brs = mp.tile([128, TCH], F32, tag="brs")
                nc.vector.tensor_copy(brs[0:BW[n], :], brp[0:BW[n], :])
                nc.vector.tensor_mul(at_sb[0:BW[n], n, :],
                                     at_ps[0:BW[n], :], brs[0:BW[n], :])

            normalize(0, atps[0])
            a2 = mps.tile([128, TCH], F32, tag="sp")
            nc.tensor.matmul(a2[0:ATW[2], :], vps[(2, 3)][:, 0:ATW[2]],
                             es[3][:], start=True, stop=True)
            normalize(1, atps[1])
            normalize(2, a2)
            # bias ones-row (T2 row 64) — after normalize(2)'s mul zeroed it
            nc.gpsimd.memset(at_sb[64:65, 2, :], 1.0)

            if pending_out is not None:
                emit_out(pending_out)
            pending_out = (b, c, at_sb)

        emit_out(pending_out)

        mps.release()
        ap.release()
        ofp.release()
        mp.release()
        qtp.release()
        esp.release()
        cinp.release()
        pp.release()
        cp.release()

    split_waits(nc)
    return nc


def split_waits(nc, max_waits=1):
    """neuronxcc walrus accepts at most one attached sync wait per
    instruction; hoist extras onto standalone event-semaphore waits."""
    n_new = 0
    for f in nc.m.functions:
        for blk in f.blocks:
            new = []
            changed = False
            for inst in blk.instructions:
                si = inst.sync_info
                ow = list(si.on_wait) if (si is not None and si.on_wait) else []
                if len(ow) > max_waits:
                    for w in ow[:-max_waits]:
                        ev = mybir.InstEventSemaphore(
                            name=f"I-waitsplit-{n_new}", ins=[], outs=[])
                        ev.engine = inst.engine
                        ev.sync_info = mybir.SyncInfo(on_wait=[w], on_update=[])
                        nc.register_instruction(ev)
                        new.append(ev)
                        n_new += 1
                    inst.sync_info = mybir.SyncInfo(
                        on_wait=ow[-max_waits:], on_update=list(si.on_update))
                    changed = True
                new.append(inst)
            if changed:
                blk.instructions = new


_GRAPH = None


def _prep_in_maps(inputs):
    inputs = {k: np.asarray(v, dtype=np.float32) for k, v in inputs.items()}
    consts = _build_host_consts(inputs)
    ctx = np.asarray(inputs["context"])           # [16, 4096, 768] f32
    audio = np.asarray(inputs["audio_context"])   # [16, 1, 1024] f32

    # pack ctx so each [128, 6, 512] chunk tile is contiguous in DRAM
    ctx16 = ctx.astype(BF)
    ctxp = np.ascontiguousarray(
        ctx16.transpose(0, 2, 1).reshape(B, 6, 128, NCH, TCH)
        .transpose(0, 3, 2, 1, 4))                # [B, NCH, 128, 6, TCH]
    apad = np.zeros((B, AUD + 2 * PAD), np.float32)
    apad[:, PAD:PAD + AUD] = audio[:, 0, :]
    a_im = np.empty((B, KS, AUD), np.float32)
    for k in range(KS):
        a_im[:, k, :] = apad[:, k:k + AUD]
    a_im = a_im.astype(BF)

    in_maps = []
    for core in range(NCORES):
        m = dict(consts)
        s = slice(core * BP, (core + 1) * BP)
        m["ctxp"] = ctxp[s]
        m["a_im"] = a_im[s]
        in_maps.append(m)
    return in_maps


def kernel(**inputs):
    global _GRAPH
    if _GRAPH is None:
        _GRAPH = _build_graph()
    nc = _GRAPH

    in_maps = _prep_in_maps(inputs)
    res = run_bass_kernel_spmd(nc, in_maps, list(range(NCORES)))
    out = np.concatenate([res.results[i]["out"] for i in range(NCORES)], axis=0)
    return out.astype(np.float32)
